# revision 9
# baseline (speedup 1.0000x reference)
"""Trainium2 Bass kernel for nn_CaptionModel_53240414601810 (diverse beam-search step).

Strategy (data-parallel over batch, 16 batches/core on 8 cores):
  reference semantics == per-batch global top-16 over s[k,v] = lp[k,v] - 0.5*counts[v] + bsum[k]
  (per-beam top-K then global top-K over K*K candidates is equivalent to a
   global top-K over all (k,v); ties resolve to ascending vocab/candidate
   index, which the block/marks machinery below reproduces exactly).

  Phase 1  (streaming): chunked scan of lp; PE replicates -0.5*counts to all
           128 (b,k)-rows via a one-hot bf16 matmul; DVE adds and computes
           64-wide block maxima -> bmax [128 rows, 786 blocks].
  Phase 2  top-16 blocks per row via max8 + match_replace "marks" trick
           (exact under duplicated values); indirect-DMA gathers the selected
           blocks of lp and counts (one [P,1]-index call per slot).
  Phase 3  per-row top-16 of the 1024 gathered candidates (marks trick),
           payload extraction via is_equal+mult+accum; pack per-batch.
  Final    per-batch top-16 of 256 candidates, exact stable ordering via
           rank-by-counting; beam permutation applied with a one-hot fp32
           PE matmul (exact) to state / beam_seq / beam_seq_logprobs.
"""
import sys

sys.path.insert(0, "/opt/trn_rl_repo")

import functools
from contextlib import ExitStack

import numpy as np
import ml_dtypes

import concourse.bass as bass
import concourse.tile as tile
from concourse import bacc
from concourse import mybir
from concourse.bass_utils import run_bass_kernel_spmd
from concourse.masks import make_identity

F32 = mybir.dt.float32
I32 = mybir.dt.int32
U32 = mybir.dt.uint32
BF16 = mybir.dt.bfloat16
AX = mybir.AxisListType
OP = mybir.AluOpType

# problem constants
B, K, V, T, H = 128, 16, 50257, 20, 512
NCORES = 8
BL = B // NCORES          # batches per core = 16
ROWS = BL * K             # (b,k) rows per core = 256
NT = ROWS // 128          # row tiles = 2
BPT = 128 // K            # batches per row tile = 8
BS = 64                   # gather block size (one 256B descriptor)
NB = (V + BS - 1) // BS   # blocks per row = 786
VP = NB * BS              # padded row length = 50304
VC = 2048                 # scan chunk width (32 blocks)
JW = 16 * BS              # gathered candidates per row = 1024
NC16 = 16 * K             # final candidates per batch = 256
LAM = 0.5
EOSP = 1000.0
NEGF = -1.0e30
REPL = -3.0e38
K0 = 1.0e6                # descending-iota base for the marks trick
SH4 = 4 * H


def _chunks():
    out = []
    c0 = 0
    while c0 < VP:
        vc = min(VC, VP - c0)
        out.append((c0, vc))
        c0 += vc
    return out


def _marks_top16(nc, pool, tiny, src, width, desc_f, tag):
    """Exact top-16 positions of src [128|16, width] -> (pos16_f ascending
    within each 8-group).  Duplicated values are handled exactly: the j-th
    needle copy claims the j-th earliest stream occurrence."""
    P = src.shape[0]
    v8a = tiny.tile([P, 8], F32, tag=f"{tag}v8a")
    nc.vector.max(v8a[:], src[:])
    s2 = pool.tile([P, width], F32, tag=f"{tag}s2")
    nc.vector.match_replace(s2[:], v8a[:], src[:], REPL)
    v8b = tiny.tile([P, 8], F32, tag=f"{tag}v8b")
    nc.vector.max(v8b[:], s2[:])
    s3 = pool.tile([P, width], F32, tag=f"{tag}s3")
    nc.vector.match_replace(s3[:], v8b[:], s2[:], REPL)

    pos16 = tiny.tile([P, 16], F32, tag=f"{tag}pos")
    pv = pool.tile([P, width], F32, tag=f"{tag}pv")
    p8 = tiny.tile([P, 8], F32, tag=f"{tag}p8")
    # positions replaced in round A: s2 == REPL there (real values never equal REPL)
    nc.vector.scalar_tensor_tensor(pv[:], s2[:], REPL, desc_f[:P, :width],
                                   op0=OP.is_equal, op1=OP.mult)
    nc.vector.max(p8[:], pv[:])
    nc.vector.tensor_scalar(pos16[:, 0:8], p8[:], -1.0, K0, op0=OP.mult, op1=OP.add)
    pvb = pool.tile([P, width], F32, tag=f"{tag}pvb")
    p8b = tiny.tile([P, 8], F32, tag=f"{tag}p8b")
    nc.vector.scalar_tensor_tensor(pvb[:], s3[:], REPL, desc_f[:P, :width],
                                   op0=OP.is_equal, op1=OP.mult)
    nc.vector.tensor_tensor(pvb[:], pvb[:], pv[:], op=OP.subtract)
    nc.vector.max(p8b[:], pvb[:])
    nc.vector.tensor_scalar(pos16[:, 8:16], p8b[:], -1.0, K0, op0=OP.mult, op1=OP.add)
    return pos16


def build_program(t_cur: int) -> bass.Bass:
    nc = bacc.Bacc("TRN2", target_bir_lowering=False)
    lp = nc.dram_tensor("lp", [ROWS, VP], F32, kind="ExternalInput")
    lpa = nc.dram_tensor("lpa", [ROWS, VP], F32, kind="ExternalInput")
    bsum = nc.dram_tensor("bsum", [128, NT], F32, kind="ExternalInput")
    seq = nc.dram_tensor("seq", [ROWS, T], F32, kind="ExternalInput")
    seqlp = nc.dram_tensor("seqlp", [ROWS, T], F32, kind="ExternalInput")
    state = nc.dram_tensor("state", [ROWS, SH4], F32, kind="ExternalInput")
    bloc16 = nc.dram_tensor("bloc16", [128, 1], F32, kind="ExternalInput")
    ltm = nc.dram_tensor("ltm", [BL, 16 * 16], F32, kind="ExternalInput")

    o_seq = nc.dram_tensor("o_seq", [ROWS, T], I32, kind="ExternalOutput")
    o_slp = nc.dram_tensor("o_slp", [ROWS, T], F32, kind="ExternalOutput")
    o_selp = nc.dram_tensor("o_selp", [BL, K], F32, kind="ExternalOutput")
    o_state = nc.dram_tensor("o_state", [ROWS, SH4], F32, kind="ExternalOutput")

    lpa_blk = lpa[:].rearrange("r (nb bs) -> (r nb) bs", bs=BS)
    lp_flat = lp[:].rearrange("r v -> (r v)").rearrange("(x o) -> x o", o=1)
    lpa_flat = lpa[:].rearrange("r v -> (r v)").rearrange("(x o) -> x o", o=1)

    with tile.TileContext(nc) as tc, ExitStack() as ctx:
        const = ctx.enter_context(tc.tile_pool(name="const", bufs=1))
        io = ctx.enter_context(tc.tile_pool(name="io", bufs=1))
        stream = ctx.enter_context(tc.tile_pool(name="stream", bufs=4))
        mid = ctx.enter_context(tc.tile_pool(name="mid", bufs=1))
        tiny = ctx.enter_context(tc.tile_pool(name="tiny", bufs=2))
        fpsum = ctx.enter_context(tc.tile_pool(name="fpsum", bufs=2, space="PSUM"))

        # ---------- constants ----------
        ident = const.tile([128, 128], F32)
        make_identity(nc, ident[:])
        bsum_t = const.tile([128, NT], F32)
        nc.sync.dma_start(bsum_t[:], bsum[:])
        bloc_t = const.tile([128, 1], F32)
        nc.sync.dma_start(bloc_t[:], bloc16[:])
        ltm_t = const.tile([BL, 256], F32)
        nc.sync.dma_start(ltm_t[:], ltm[:])

        def iota_f(shape, pattern, base, cm, tag):
            ti = const.tile(shape, I32, tag=tag + "i")
            nc.gpsimd.iota(ti[:], pattern=pattern, base=base, channel_multiplier=cm)
            tf = const.tile(shape, F32, tag=tag + "f")
            nc.vector.tensor_copy(tf[:], ti[:])
            return tf

        pidx_f = iota_f([128, 1], [[0, 1]], 0, 1, "pidx")
        iota16f = iota_f([128, 16], [[1, 16]], 0, 0, "io16")
        descNB = iota_f([128, NB], [[-1, NB]], int(K0), 0, "dnb")
        iotaJW = iota_f([128, JW], [[1, JW]], 0, 0, "ijw")
        descJW = iota_f([128, JW], [[-1, JW]], int(K0), 0, "djw")
        iotaNC = iota_f([BL, NC16], [[1, NC16]], 0, 0, "inc")
        descNC = iota_f([BL, NC16], [[-1, NC16]], int(K0), 0, "dnc")
        rowbase = []
        rowbaseE = []
        for rt in range(NT):
            rowbase.append(iota_f([128, 1], [[0, 1]], rt * 128 * NB, NB, f"rb{rt}"))
            rowbaseE.append(iota_f([128, 1], [[0, 1]], rt * 128 * VP, VP, f"re{rt}"))

        # ---------- resident inputs ----------
        seq_sb, seqlp_sb, state_sb = [], [], []
        for rt in range(NT):
            s_t = io.tile([128, T], F32, tag=f"seq{rt}")
            nc.sync.dma_start(s_t[:], seq[rt * 128:(rt + 1) * 128, :])
            seq_sb.append(s_t)
            sl_t = io.tile([128, T], F32, tag=f"seqlp{rt}")
            nc.sync.dma_start(sl_t[:], seqlp[rt * 128:(rt + 1) * 128, :])
            seqlp_sb.append(sl_t)
            st_t = io.tile([128, SH4], F32, tag=f"state{rt}")
            nc.sync.dma_start(st_t[:], state[rt * 128:(rt + 1) * 128, :])
            state_sb.append(st_t)

        # per-batch packed candidates
        C_s = io.tile([BL, NC16], F32, tag="C_s")
        C_t = io.tile([BL, NC16], F32, tag="C_t")

        # ---------- per row-tile scan & candidate extraction ----------
        for rt in range(NT):
            bmax = io.tile([128, NB], F32, tag=f"bmax{rt}")
            nbdone = 0
            for (c0, vc) in _chunks():
                lp_t = stream.tile([128, VC], F32, tag="lp")
                nc.sync.dma_start(lp_t[:, :vc], lpa[rt * 128:(rt + 1) * 128, c0:c0 + vc])
                nblk_c = vc // BS
                nc.vector.tensor_reduce(
                    bmax[:, nbdone:nbdone + nblk_c],
                    lp_t[:, :vc].rearrange("p (nb bs) -> p nb bs", bs=BS),
                    axis=AX.X, op=OP.max)
                nbdone += nblk_c

            nc.vector.tensor_scalar(bmax[:], bmax[:], bsum_t[:, rt:rt + 1], None, op0=OP.add)

            # top-16 blocks per row
            bid16 = _marks_top16(nc, mid, tiny, bmax, NB, descNB, "blk")

            # gather the selected blocks of lp and counts
            idxl_f = tiny.tile([128, 16], F32, tag="idxlf")
            nc.vector.tensor_scalar(idxl_f[:], bid16[:], rowbase[rt][:, 0:1], None, op0=OP.add)
            idxl_i = tiny.tile([128, 16], I32, tag="idxli")
            nc.vector.tensor_copy(idxl_i[:], idxl_f[:])
            lp_g = mid.tile([128, 16, BS], F32, tag="lp_g")
            for j in range(16):
                nc.gpsimd.indirect_dma_start(
                    out=lp_g[:, j, :], out_offset=None, in_=lpa_blk,
                    in_offset=bass.IndirectOffsetOnAxis(ap=idxl_i[:, j:j + 1], axis=0))
            lp_gf = lp_g[:].rearrange("p j b -> p (j b)")

            s_g = mid.tile([128, JW], F32, tag="s_g")
            nc.vector.tensor_scalar(s_g[:], lp_gf, bsum_t[:, rt:rt + 1], None, op0=OP.add)

            # per-row top-16 candidates
            J16 = _marks_top16(nc, mid, tiny, s_g, JW, descJW, "cand")

            # vocab id: block_id[slot]*64 + (J16 & 63),  slot = J16 >> 6
            J16u = tiny.tile([128, 16], U32, tag="J16u")
            nc.vector.tensor_copy(J16u[:], J16[:])
            slot_u = tiny.tile([128, 16], U32, tag="slotu")
            nc.vector.tensor_scalar(slot_u[:], J16u[:], 6, None, op0=OP.logical_shift_right)
            slot_f = tiny.tile([128, 16], F32, tag="slotf")
            nc.vector.tensor_copy(slot_f[:], slot_u[:])
            e_f = tiny.tile([128, 16], F32, tag="ef")
            nc.vector.scalar_tensor_tensor(e_f[:], slot_f[:], -64.0, J16[:],
                                           op0=OP.mult, op1=OP.add)
            bsel = tiny.tile([128, 16], F32, tag="bsel")
            scr16 = tiny.tile([128, 16], F32, tag="scr16")
            for r in range(16):
                nc.vector.scalar_tensor_tensor(
                    scr16[:], iota16f[:], slot_f[:, r:r + 1], bid16[:],
                    op0=OP.is_equal, op1=OP.mult, accum_out=bsel[:, r:r + 1])
            tok16 = tiny.tile([128, 16], F32, tag="tok16")
            nc.vector.scalar_tensor_tensor(tok16[:], bsel[:], float(BS), e_f[:],
                                           op0=OP.mult, op1=OP.add)

            # candidate s values via a tiny element gather from lpa (+bsum)
            idxv_f = tiny.tile([128, 16], F32, tag="idxvf")
            nc.vector.tensor_scalar(idxv_f[:], tok16[:], rowbaseE[rt][:, 0:1], None, op0=OP.add)
            idxv_i = tiny.tile([128, 16], I32, tag="idxvi")
            nc.vector.tensor_copy(idxv_i[:], idxv_f[:])
            vg = tiny.tile([128, 16], F32, tag="vg")
            for r in range(16):
                nc.gpsimd.indirect_dma_start(
                    out=vg[:, r:r + 1], out_offset=None, in_=lpa_flat,
                    in_offset=bass.IndirectOffsetOnAxis(ap=idxv_i[:, r:r + 1], axis=0))
            V16 = tiny.tile([128, 16], F32, tag="V16")
            nc.vector.tensor_scalar(V16[:], vg[:], bsum_t[:, rt:rt + 1], None, op0=OP.add)

            # pack to per-batch layout
            nc.sync.dma_start(C_s[rt * BPT:(rt + 1) * BPT, :], V16[:])
            nc.sync.dma_start(C_t[rt * BPT:(rt + 1) * BPT, :], tok16[:])

        # ---------- final per-batch top-16 ----------
        j16b = _marks_top16(nc, mid, tiny, C_s, NC16, descNC, "fin")
        W_s = tiny.tile([BL, 16], F32, tag="W_s")
        W_t = tiny.tile([BL, 16], F32, tag="W_t")
        scrF = mid.tile([BL, NC16], F32, tag="scrF")
        scrG = mid.tile([BL, NC16], F32, tag="scrG")
        for r in range(16):
            nc.vector.scalar_tensor_tensor(
                scrF[:], iotaNC[:], j16b[:, r:r + 1], C_s[:],
                op0=OP.is_equal, op1=OP.mult, accum_out=W_s[:, r:r + 1])
            nc.vector.scalar_tensor_tensor(
                scrG[:], iotaNC[:], j16b[:, r:r + 1], C_t[:],
                op0=OP.is_equal, op1=OP.mult, accum_out=W_t[:, r:r + 1])
        # source beam q = j16b >> 4
        j16bu = tiny.tile([BL, 16], U32, tag="j16bu")
        nc.vector.tensor_copy(j16bu[:], j16b[:])
        q_u = tiny.tile([BL, 16], U32, tag="qu")
        nc.vector.tensor_scalar(q_u[:], j16bu[:], 4, None, op0=OP.logical_shift_right)
        W_q = tiny.tile([BL, 16], F32, tag="W_q")
        nc.vector.tensor_copy(W_q[:], q_u[:])

        # exact stable descending order: rank-by-counting
        ones16 = const.tile([BL, 16], F32, tag="ones16")
        nc.vector.memset(ones16[:], 1.0)
        GT = tiny.tile([BL, 16], F32, tag="GT")
        EQ = tiny.tile([BL, 16], F32, tag="EQ")
        scr16b = tiny.tile([BL, 16], F32, tag="scr16b")
        for i in range(16):
            nc.vector.scalar_tensor_tensor(
                scr16b[:], W_s[:], W_s[:, i:i + 1], ones16[:],
                op0=OP.is_gt, op1=OP.mult, accum_out=GT[:, i:i + 1])
            nc.vector.scalar_tensor_tensor(
                scr16b[:], W_s[:], W_s[:, i:i + 1], ltm_t[:, i * 16:(i + 1) * 16],
                op0=OP.is_equal, op1=OP.mult, accum_out=EQ[:, i:i + 1])
        RANK = tiny.tile([BL, 16], F32, tag="RANK")
        nc.vector.tensor_tensor(RANK[:], GT[:], EQ[:], op=OP.add)

        SELP = tiny.tile([BL, 16], F32, tag="SELP")
        TOK = tiny.tile([BL, 16], F32, tag="TOK")
        QQ = tiny.tile([BL, 16], F32, tag="QQ")
        for r in range(16):
            for src_t, dst_t in ((W_s, SELP), (W_t, TOK), (W_q, QQ)):
                nc.vector.scalar_tensor_tensor(
                    scr16b[:], RANK[:], float(r), src_t[:],
                    op0=OP.is_equal, op1=OP.mult, accum_out=dst_t[:, r:r + 1])

        nc.sync.dma_start(o_selp[:], SELP[:])

        # ---------- apply beam permutation (state / seq / seqlp) ----------
        for rt in range(NT):
            qg = tiny.tile([128, 1], F32, tag="qg")
            nc.sync.dma_start(qg[:], QQ[rt * BPT:(rt + 1) * BPT, :])
            nc.vector.tensor_tensor(qg[:], qg[:], bloc_t[:], op=OP.add)
            tpq = fpsum.tile([128, 512], F32, tag="fin")
            nc.tensor.transpose(tpq[:, 0:128], qg[:].to_broadcast([128, 128]), ident[:])
            qg_tr = mid.tile([128, 128], F32, tag="qg_tr")
            nc.vector.tensor_copy(qg_tr[:], tpq[:, 0:128])
            lhsT = mid.tile([128, 128], F32, tag="lhsT")
            nc.vector.tensor_scalar(lhsT[:], qg_tr[:], pidx_f[:, 0:1], None, op0=OP.is_equal)

            st_out = mid.tile([128, SH4], F32, tag="st_out")
            for s0 in range(0, SH4, 512):
                pst = fpsum.tile([128, 512], F32, tag="fin")
                nc.tensor.matmul(pst[:], lhsT=lhsT[:], rhs=state_sb[rt][:, s0:s0 + 512],
                                 start=True, stop=True)
                nc.scalar.copy(st_out[:, s0:s0 + 512], pst[:])
            nc.sync.dma_start(o_state[rt * 128:(rt + 1) * 128, :], st_out[:])

            psq = fpsum.tile([128, 512], F32, tag="fin")
            nc.tensor.matmul(psq[:, 0:T], lhsT=lhsT[:], rhs=seq_sb[rt][:], start=True, stop=True)
            psl = fpsum.tile([128, 512], F32, tag="fin")
            nc.tensor.matmul(psl[:, 0:T], lhsT=lhsT[:], rhs=seqlp_sb[rt][:], start=True, stop=True)

            tok128 = tiny.tile([128, 1], F32, tag="tok128")
            nc.sync.dma_start(tok128[:], TOK[rt * BPT:(rt + 1) * BPT, :])
            idxr_f = tiny.tile([128, 1], F32, tag="idxrf")
            nc.vector.tensor_scalar(idxr_f[:], qg[:], float(rt * 128), float(VP),
                                    op0=OP.add, op1=OP.mult)
            nc.vector.tensor_tensor(idxr_f[:], idxr_f[:], tok128[:], op=OP.add)
            idxr_i = tiny.tile([128, 1], I32, tag="idxri")
            nc.vector.tensor_copy(idxr_i[:], idxr_f[:])
            r128 = tiny.tile([128, 1], F32, tag="r128")
            nc.gpsimd.indirect_dma_start(
                out=r128[:], out_offset=None, in_=lp_flat,
                in_offset=bass.IndirectOffsetOnAxis(ap=idxr_i[:], axis=0))

            oseq = mid.tile([128, T], I32, tag="oseq")
            if t_cur > 0:
                nc.vector.tensor_copy(oseq[:, 0:t_cur], psq[:, 0:t_cur])
            nc.vector.tensor_copy(oseq[:, t_cur:t_cur + 1], tok128[:])
            if t_cur + 1 < T:
                nc.vector.tensor_copy(oseq[:, t_cur + 1:], seq_sb[rt][:, t_cur + 1:])
            nc.sync.dma_start(o_seq[rt * 128:(rt + 1) * 128, :], oseq[:])

            oslp = mid.tile([128, T], F32, tag="oslp")
            if t_cur > 0:
                nc.vector.tensor_copy(oslp[:, 0:t_cur], psl[:, 0:t_cur])
            nc.vector.tensor_copy(oslp[:, t_cur:t_cur + 1], r128[:])
            if t_cur + 1 < T:
                nc.vector.tensor_copy(oslp[:, t_cur + 1:], seqlp_sb[rt][:, t_cur + 1:])
            nc.sync.dma_start(o_slp[rt * 128:(rt + 1) * 128, :], oslp[:])

    nc.finalize()
    return nc


@functools.lru_cache(maxsize=4)
def _program(t_cur: int) -> bass.Bass:
    return build_program(t_cur)


def _host_counts(prev_decisions: np.ndarray) -> np.ndarray:
    tok = prev_decisions.reshape(B, -1).astype(np.int64)
    counts = np.zeros((B, V), np.float32)
    np.add.at(counts, (np.arange(B)[:, None], tok), 1.0)
    return counts


def kernel(**inputs) -> tuple:
    logprobs = np.asarray(inputs["logprobs"], dtype=np.float32)
    beam_seq = np.asarray(inputs["beam_seq"])
    beam_seq_logprobs = np.asarray(inputs["beam_seq_logprobs"], dtype=np.float32)
    beam_logprobs_sum = np.asarray(inputs["beam_logprobs_sum"], dtype=np.float32)
    state = np.asarray(inputs["state"], dtype=np.float32)
    prev_decisions = np.asarray(inputs["prev_decisions"])
    t_cur = int(np.asarray(inputs["t"]))

    nc = _program(t_cur)
    counts = _host_counts(prev_decisions)

    # shared constant tables
    p = np.arange(128)
    bloc_np = ((p // K) * K).astype(np.float32).reshape(128, 1)
    ltm_np = np.zeros((BL, 16, 16), np.float32)
    for i in range(16):
        ltm_np[:, i, :i] = 1.0
    ltm_np = ltm_np.reshape(BL, 256)

    in_maps = []
    for c in range(NCORES):
        bsl = slice(c * BL, (c + 1) * BL)
        lp_pad = np.full((ROWS, VP), NEGF, np.float32)
        lp_pad[:, :V] = logprobs[bsl].reshape(ROWS, V)
        lp_pad[:, V - 1] -= EOSP          # unaug (EOS-penalized) raw logprobs
        # diversity penalty applied sparsely (aug): lpa = lp - 0.5*counts
        lpa = lp_pad.copy()
        lpa_v = lpa.reshape(BL, K, VP)
        for b in range(BL):
            toks = np.unique(prev_decisions[c * BL + b].reshape(-1).astype(np.int64))
            lpa_v[b, :, toks] = (lpa_v[b, :, toks]
                                 - (LAM * counts[c * BL + b, toks])[:, None])
        bsum_sh = beam_logprobs_sum[bsl]               # [BL, K]
        bsum_np = np.stack([bsum_sh[rt * BPT:(rt + 1) * BPT].reshape(128)
                            for rt in range(NT)], axis=1).astype(np.float32)
        seq_np = np.ascontiguousarray(
            beam_seq[bsl].astype(np.float32).transpose(0, 2, 1).reshape(ROWS, T))
        seqlp_np = np.ascontiguousarray(
            beam_seq_logprobs[bsl].transpose(0, 2, 1).reshape(ROWS, T))
        state_np = np.ascontiguousarray(
            state[:, :, bsl].transpose(2, 3, 0, 1, 4).reshape(ROWS, SH4))
        in_maps.append({
            "lp": lp_pad,
            "lpa": lpa,
            "bsum": bsum_np,
            "seq": seq_np,
            "seqlp": seqlp_np,
            "state": state_np,
            "bloc16": bloc_np,
            "ltm": ltm_np,
        })

    res = run_bass_kernel_spmd(nc, in_maps, core_ids=list(range(NCORES)))

    new_seq = np.zeros((B, T, K), np.float32)
    new_slp = np.zeros((B, T, K), np.float32)
    sel_p = np.zeros((B, K), np.float32)
    new_state = np.zeros((2, 2, B, K, H), np.float32)
    for c in range(NCORES):
        r = res.results[c]
        bsl = slice(c * BL, (c + 1) * BL)
        new_seq[bsl] = r["o_seq"].reshape(BL, K, T).transpose(0, 2, 1)
        new_slp[bsl] = r["o_slp"].reshape(BL, K, T).transpose(0, 2, 1)
        sel_p[bsl] = r["o_selp"].reshape(BL, K)
        new_state[:, :, bsl] = (
            r["o_state"].reshape(BL, K, 2, 2, H).transpose(2, 3, 0, 1, 4))
    return (
        new_seq.astype(beam_seq.dtype),
        new_slp.astype(np.float32),
        sel_p.astype(np.float32),
        new_state.astype(np.float32),
    )


# revision 10
# speedup vs baseline: 1.1352x; 1.1352x over previous
"""Trainium2 Bass kernel for nn_CaptionModel_53240414601810 (diverse beam-search step).

Strategy (data-parallel over batch, 16 batches/core on 8 cores):
  reference semantics == per-batch global top-16 over s[k,v] = lp[k,v] - 0.5*counts[v] + bsum[k]
  (per-beam top-K then global top-K over K*K candidates is equivalent to a
   global top-K over all (k,v); ties resolve to ascending vocab/candidate
   index, which the block/marks machinery below reproduces exactly).

  Phase 1  (streaming): chunked scan of lp; PE replicates -0.5*counts to all
           128 (b,k)-rows via a one-hot bf16 matmul; DVE adds and computes
           64-wide block maxima -> bmax [128 rows, 786 blocks].
  Phase 2  top-16 blocks per row via max8 + match_replace "marks" trick
           (exact under duplicated values); indirect-DMA gathers the selected
           blocks of lp and counts (one [P,1]-index call per slot).
  Phase 3  per-row top-16 of the 1024 gathered candidates (marks trick),
           payload extraction via is_equal+mult+accum; pack per-batch.
  Final    per-batch top-16 of 256 candidates, exact stable ordering via
           rank-by-counting; beam permutation applied with a one-hot fp32
           PE matmul (exact) to state / beam_seq / beam_seq_logprobs.
"""
import sys

sys.path.insert(0, "/opt/trn_rl_repo")

import functools
from contextlib import ExitStack

import numpy as np
import ml_dtypes

import concourse.bass as bass
import concourse.tile as tile
from concourse import bacc
from concourse import mybir
from concourse.bass_utils import run_bass_kernel_spmd
from concourse.masks import make_identity

F32 = mybir.dt.float32
I32 = mybir.dt.int32
U32 = mybir.dt.uint32
BF16 = mybir.dt.bfloat16
AX = mybir.AxisListType
OP = mybir.AluOpType

# problem constants
B, K, V, T, H = 128, 16, 50257, 20, 512
NCORES = 8
BL = B // NCORES          # batches per core = 16
ROWS = BL * K             # (b,k) rows per core = 256
NT = ROWS // 128          # row tiles = 2
BPT = 128 // K            # batches per row tile = 8
BS = 64                   # gather block size (one 256B descriptor)
NB = (V + BS - 1) // BS   # blocks per row = 786
VP = NB * BS              # padded row length = 50304
VC = 2048                 # scan chunk width (32 blocks)
JW = 16 * BS              # gathered candidates per row = 1024
NC16 = 16 * K             # final candidates per batch = 256
LAM = 0.5
EOSP = 1000.0
NEGF = -1.0e30
REPL = -3.0e38
K0 = 1.0e6                # descending-iota base for the marks trick
SH4 = 4 * H


def _chunks():
    out = []
    c0 = 0
    while c0 < VP:
        vc = min(VC, VP - c0)
        out.append((c0, vc))
        c0 += vc
    return out


def _marks_top16(nc, pool, tiny, src, width, desc_f, tag):
    """Exact top-16 positions of src [128|16, width] -> (pos16_f ascending
    within each 8-group).  Duplicated values are handled exactly: the j-th
    needle copy claims the j-th earliest stream occurrence."""
    P = src.shape[0]
    v8a = tiny.tile([P, 8], F32, tag=f"{tag}v8a")
    nc.vector.max(v8a[:], src[:])
    s2 = pool.tile([P, width], F32, tag=f"{tag}s2")
    nc.vector.match_replace(s2[:], v8a[:], src[:], REPL)
    v8b = tiny.tile([P, 8], F32, tag=f"{tag}v8b")
    nc.vector.max(v8b[:], s2[:])
    s3 = pool.tile([P, width], F32, tag=f"{tag}s3")
    nc.vector.match_replace(s3[:], v8b[:], s2[:], REPL)

    pos16 = tiny.tile([P, 16], F32, tag=f"{tag}pos")
    pv = pool.tile([P, width], F32, tag=f"{tag}pv")
    p8 = tiny.tile([P, 8], F32, tag=f"{tag}p8")
    # positions replaced in round A: s2 == REPL there (real values never equal REPL)
    nc.vector.scalar_tensor_tensor(pv[:], s2[:], REPL, desc_f[:P, :width],
                                   op0=OP.is_equal, op1=OP.mult)
    nc.vector.max(p8[:], pv[:])
    nc.vector.tensor_scalar(pos16[:, 0:8], p8[:], -1.0, K0, op0=OP.mult, op1=OP.add)
    pvb = pool.tile([P, width], F32, tag=f"{tag}pvb")
    p8b = tiny.tile([P, 8], F32, tag=f"{tag}p8b")
    nc.vector.scalar_tensor_tensor(pvb[:], s3[:], REPL, desc_f[:P, :width],
                                   op0=OP.is_equal, op1=OP.mult)
    nc.vector.tensor_tensor(pvb[:], pvb[:], pv[:], op=OP.subtract)
    nc.vector.max(p8b[:], pvb[:])
    nc.vector.tensor_scalar(pos16[:, 8:16], p8b[:], -1.0, K0, op0=OP.mult, op1=OP.add)
    return pos16


def build_program(t_cur: int) -> bass.Bass:
    nc = bacc.Bacc("TRN2", target_bir_lowering=False)
    lp = nc.dram_tensor("lp", [ROWS, VP], F32, kind="ExternalInput")
    lpa = nc.dram_tensor("lpa", [ROWS, VP], F32, kind="ExternalInput")
    bsum = nc.dram_tensor("bsum", [128, NT], F32, kind="ExternalInput")
    seq = nc.dram_tensor("seq", [ROWS, T], F32, kind="ExternalInput")
    seqlp = nc.dram_tensor("seqlp", [ROWS, T], F32, kind="ExternalInput")
    state = nc.dram_tensor("state", [ROWS, SH4], F32, kind="ExternalInput")
    bloc16 = nc.dram_tensor("bloc16", [128, 1], F32, kind="ExternalInput")
    ltm = nc.dram_tensor("ltm", [BL, 16 * 16], F32, kind="ExternalInput")

    o_seq = nc.dram_tensor("o_seq", [ROWS, T], I32, kind="ExternalOutput")
    o_slp = nc.dram_tensor("o_slp", [ROWS, T], F32, kind="ExternalOutput")
    o_selp = nc.dram_tensor("o_selp", [BL, K], F32, kind="ExternalOutput")
    o_state = nc.dram_tensor("o_state", [ROWS, SH4], F32, kind="ExternalOutput")

    lpa_blk = lpa[:].rearrange("r (nb bs) -> (r nb) bs", bs=BS)
    lp_flat = lp[:].rearrange("r v -> (r v)").rearrange("(x o) -> x o", o=1)
    lpa_flat = lpa[:].rearrange("r v -> (r v)").rearrange("(x o) -> x o", o=1)

    with tile.TileContext(nc) as tc, ExitStack() as ctx:
        const = ctx.enter_context(tc.tile_pool(name="const", bufs=1))
        io = ctx.enter_context(tc.tile_pool(name="io", bufs=1))
        stream = ctx.enter_context(tc.tile_pool(name="stream", bufs=4))
        mid = ctx.enter_context(tc.tile_pool(name="mid", bufs=1))
        tiny = ctx.enter_context(tc.tile_pool(name="tiny", bufs=2))
        fpsum = ctx.enter_context(tc.tile_pool(name="fpsum", bufs=2, space="PSUM"))

        # ---------- constants ----------
        ident = const.tile([128, 128], F32)
        make_identity(nc, ident[:])
        bsum_t = const.tile([128, NT], F32)
        nc.sync.dma_start(bsum_t[:], bsum[:])
        bloc_t = const.tile([128, 1], F32)
        nc.sync.dma_start(bloc_t[:], bloc16[:])
        ltm_t = const.tile([BL, 256], F32)
        nc.sync.dma_start(ltm_t[:], ltm[:])

        def iota_f(shape, pattern, base, cm, tag):
            ti = const.tile(shape, I32, tag=tag + "i")
            nc.gpsimd.iota(ti[:], pattern=pattern, base=base, channel_multiplier=cm)
            tf = const.tile(shape, F32, tag=tag + "f")
            nc.vector.tensor_copy(tf[:], ti[:])
            return tf

        pidx_f = iota_f([128, 1], [[0, 1]], 0, 1, "pidx")
        iota16f = iota_f([128, 16], [[1, 16]], 0, 0, "io16")
        descNB = iota_f([128, NB], [[-1, NB]], int(K0), 0, "dnb")
        iotaJW = iota_f([128, JW], [[1, JW]], 0, 0, "ijw")
        descJW = iota_f([128, JW], [[-1, JW]], int(K0), 0, "djw")
        iotaNC = iota_f([BL, NC16], [[1, NC16]], 0, 0, "inc")
        descNC = iota_f([BL, NC16], [[-1, NC16]], int(K0), 0, "dnc")
        rowbase = []
        rowbaseE = []
        for rt in range(NT):
            rowbase.append(iota_f([128, 1], [[0, 1]], rt * 128 * NB, NB, f"rb{rt}"))
            rowbaseE.append(iota_f([128, 1], [[0, 1]], rt * 128 * VP, VP, f"re{rt}"))

        # ---------- resident inputs ----------
        seq_sb, seqlp_sb, state_sb = [], [], []
        for rt in range(NT):
            s_t = io.tile([128, T], F32, tag=f"seq{rt}")
            nc.sync.dma_start(s_t[:], seq[rt * 128:(rt + 1) * 128, :])
            seq_sb.append(s_t)
            sl_t = io.tile([128, T], F32, tag=f"seqlp{rt}")
            nc.sync.dma_start(sl_t[:], seqlp[rt * 128:(rt + 1) * 128, :])
            seqlp_sb.append(sl_t)
            st_t = io.tile([128, SH4], F32, tag=f"state{rt}")
            nc.sync.dma_start(st_t[:], state[rt * 128:(rt + 1) * 128, :])
            state_sb.append(st_t)

        # per-batch packed candidates
        C_s = io.tile([BL, NC16], F32, tag="C_s")
        C_t = io.tile([BL, NC16], F32, tag="C_t")

        # ---------- per row-tile scan & candidate extraction (pipelined) ----------
        bmaxs, bid16s, lp_gs = [], [], []
        for rt in range(NT):
            bmax = io.tile([128, NB], F32, tag=f"bmax{rt}")
            nbdone = 0
            for (c0, vc) in _chunks():
                lp_t = stream.tile([128, VC], F32, tag="lp")
                nc.sync.dma_start(lp_t[:, :vc], lpa[rt * 128:(rt + 1) * 128, c0:c0 + vc])
                nblk_c = vc // BS
                nc.vector.tensor_reduce(
                    bmax[:, nbdone:nbdone + nblk_c],
                    lp_t[:, :vc].rearrange("p (nb bs) -> p nb bs", bs=BS),
                    axis=AX.X, op=OP.max)
                nbdone += nblk_c
            nc.vector.tensor_scalar(bmax[:], bmax[:], bsum_t[:, rt:rt + 1], None, op0=OP.add)
            bmaxs.append(bmax)

        for rt in range(NT):
            bid16 = _marks_top16(nc, mid, tiny, bmaxs[rt], NB, descNB, f"blk{rt}")
            bid16s.append(bid16)
            idxl_f = tiny.tile([128, 16], F32, tag=f"idxlf{rt}")
            nc.vector.tensor_scalar(idxl_f[:], bid16[:], rowbase[rt][:, 0:1], None, op0=OP.add)
            idxl_i = tiny.tile([128, 16], I32, tag=f"idxli{rt}")
            nc.vector.tensor_copy(idxl_i[:], idxl_f[:])
            lp_g = mid.tile([128, 16, BS], F32, tag=f"lp_g{rt}")
            for j in range(16):
                nc.gpsimd.indirect_dma_start(
                    out=lp_g[:, j, :], out_offset=None, in_=lpa_blk,
                    in_offset=bass.IndirectOffsetOnAxis(ap=idxl_i[:, j:j + 1], axis=0))
            lp_gs.append(lp_g)

        for rt in range(NT):
            bid16 = bid16s[rt]
            lp_gf = lp_gs[rt][:].rearrange("p j b -> p (j b)")
            s_g = mid.tile([128, JW], F32, tag="s_g")
            nc.vector.tensor_scalar(s_g[:], lp_gf, bsum_t[:, rt:rt + 1], None, op0=OP.add)

            # per-row top-16 candidates
            J16 = _marks_top16(nc, mid, tiny, s_g, JW, descJW, "cand")

            # vocab id: block_id[slot]*64 + (J16 & 63),  slot = J16 >> 6
            J16u = tiny.tile([128, 16], U32, tag="J16u")
            nc.vector.tensor_copy(J16u[:], J16[:])
            slot_u = tiny.tile([128, 16], U32, tag="slotu")
            nc.vector.tensor_scalar(slot_u[:], J16u[:], 6, None, op0=OP.logical_shift_right)
            slot_f = tiny.tile([128, 16], F32, tag="slotf")
            nc.vector.tensor_copy(slot_f[:], slot_u[:])
            e_f = tiny.tile([128, 16], F32, tag="ef")
            nc.vector.scalar_tensor_tensor(e_f[:], slot_f[:], -64.0, J16[:],
                                           op0=OP.mult, op1=OP.add)
            bsel = tiny.tile([128, 16], F32, tag="bsel")
            scr16 = tiny.tile([128, 16], F32, tag="scr16")
            for r in range(16):
                nc.vector.scalar_tensor_tensor(
                    scr16[:], iota16f[:], slot_f[:, r:r + 1], bid16[:],
                    op0=OP.is_equal, op1=OP.mult, accum_out=bsel[:, r:r + 1])
            tok16 = tiny.tile([128, 16], F32, tag="tok16")
            nc.vector.scalar_tensor_tensor(tok16[:], bsel[:], float(BS), e_f[:],
                                           op0=OP.mult, op1=OP.add)

            V16 = tiny.tile([128, 16], F32, tag="V16")
            scrA = mid.tile([128, JW], F32, tag="scrA")
            for r in range(16):
                nc.vector.scalar_tensor_tensor(
                    scrA[:], iotaJW[:], J16[:, r:r + 1], s_g[:],
                    op0=OP.is_equal, op1=OP.mult, accum_out=V16[:, r:r + 1])

            # pack to per-batch layout
            nc.sync.dma_start(C_s[rt * BPT:(rt + 1) * BPT, :], V16[:])
            nc.sync.dma_start(C_t[rt * BPT:(rt + 1) * BPT, :], tok16[:])

        # ---------- final per-batch top-16 ----------
        j16b = _marks_top16(nc, mid, tiny, C_s, NC16, descNC, "fin")
        W_s = tiny.tile([BL, 16], F32, tag="W_s")
        W_t = tiny.tile([BL, 16], F32, tag="W_t")
        scrF = mid.tile([BL, NC16], F32, tag="scrF")
        scrG = mid.tile([BL, NC16], F32, tag="scrG")
        for r in range(16):
            nc.vector.scalar_tensor_tensor(
                scrF[:], iotaNC[:], j16b[:, r:r + 1], C_s[:],
                op0=OP.is_equal, op1=OP.mult, accum_out=W_s[:, r:r + 1])
            nc.vector.scalar_tensor_tensor(
                scrG[:], iotaNC[:], j16b[:, r:r + 1], C_t[:],
                op0=OP.is_equal, op1=OP.mult, accum_out=W_t[:, r:r + 1])
        # source beam q = j16b >> 4
        j16bu = tiny.tile([BL, 16], U32, tag="j16bu")
        nc.vector.tensor_copy(j16bu[:], j16b[:])
        q_u = tiny.tile([BL, 16], U32, tag="qu")
        nc.vector.tensor_scalar(q_u[:], j16bu[:], 4, None, op0=OP.logical_shift_right)
        W_q = tiny.tile([BL, 16], F32, tag="W_q")
        nc.vector.tensor_copy(W_q[:], q_u[:])

        # exact stable descending order: rank-by-counting
        ones16 = const.tile([BL, 16], F32, tag="ones16")
        nc.vector.memset(ones16[:], 1.0)
        GT = tiny.tile([BL, 16], F32, tag="GT")
        EQ = tiny.tile([BL, 16], F32, tag="EQ")
        scr16b = tiny.tile([BL, 16], F32, tag="scr16b")
        for i in range(16):
            nc.vector.scalar_tensor_tensor(
                scr16b[:], W_s[:], W_s[:, i:i + 1], ones16[:],
                op0=OP.is_gt, op1=OP.mult, accum_out=GT[:, i:i + 1])
            nc.vector.scalar_tensor_tensor(
                scr16b[:], W_s[:], W_s[:, i:i + 1], ltm_t[:, i * 16:(i + 1) * 16],
                op0=OP.is_equal, op1=OP.mult, accum_out=EQ[:, i:i + 1])
        RANK = tiny.tile([BL, 16], F32, tag="RANK")
        nc.vector.tensor_tensor(RANK[:], GT[:], EQ[:], op=OP.add)

        SELP = tiny.tile([BL, 16], F32, tag="SELP")
        TOK = tiny.tile([BL, 16], F32, tag="TOK")
        QQ = tiny.tile([BL, 16], F32, tag="QQ")
        for r in range(16):
            for src_t, dst_t in ((W_s, SELP), (W_t, TOK), (W_q, QQ)):
                nc.vector.scalar_tensor_tensor(
                    scr16b[:], RANK[:], float(r), src_t[:],
                    op0=OP.is_equal, op1=OP.mult, accum_out=dst_t[:, r:r + 1])

        nc.sync.dma_start(o_selp[:], SELP[:])

        # ---------- apply beam permutation (state / seq / seqlp) ----------
        for rt in range(NT):
            qg = tiny.tile([128, 1], F32, tag="qg")
            nc.sync.dma_start(qg[:], QQ[rt * BPT:(rt + 1) * BPT, :])
            nc.vector.tensor_tensor(qg[:], qg[:], bloc_t[:], op=OP.add)
            tpq = fpsum.tile([128, 512], F32, tag="fin")
            nc.tensor.transpose(tpq[:, 0:128], qg[:].to_broadcast([128, 128]), ident[:])
            qg_tr = mid.tile([128, 128], F32, tag="qg_tr")
            nc.vector.tensor_copy(qg_tr[:], tpq[:, 0:128])
            lhsT = mid.tile([128, 128], F32, tag="lhsT")
            nc.vector.tensor_scalar(lhsT[:], qg_tr[:], pidx_f[:, 0:1], None, op0=OP.is_equal)

            st_out = mid.tile([128, SH4], F32, tag="st_out")
            for s0 in range(0, SH4, 512):
                pst = fpsum.tile([128, 512], F32, tag="fin")
                nc.tensor.matmul(pst[:], lhsT=lhsT[:], rhs=state_sb[rt][:, s0:s0 + 512],
                                 start=True, stop=True)
                nc.scalar.copy(st_out[:, s0:s0 + 512], pst[:])
            nc.sync.dma_start(o_state[rt * 128:(rt + 1) * 128, :], st_out[:])

            psq = fpsum.tile([128, 512], F32, tag="fin")
            nc.tensor.matmul(psq[:, 0:T], lhsT=lhsT[:], rhs=seq_sb[rt][:], start=True, stop=True)
            psl = fpsum.tile([128, 512], F32, tag="fin")
            nc.tensor.matmul(psl[:, 0:T], lhsT=lhsT[:], rhs=seqlp_sb[rt][:], start=True, stop=True)

            tok128 = tiny.tile([128, 1], F32, tag="tok128")
            nc.sync.dma_start(tok128[:], TOK[rt * BPT:(rt + 1) * BPT, :])
            idxr_f = tiny.tile([128, 1], F32, tag="idxrf")
            nc.vector.tensor_scalar(idxr_f[:], qg[:], float(rt * 128), float(VP),
                                    op0=OP.add, op1=OP.mult)
            nc.vector.tensor_tensor(idxr_f[:], idxr_f[:], tok128[:], op=OP.add)
            idxr_i = tiny.tile([128, 1], I32, tag="idxri")
            nc.vector.tensor_copy(idxr_i[:], idxr_f[:])
            r128 = tiny.tile([128, 1], F32, tag="r128")
            nc.gpsimd.indirect_dma_start(
                out=r128[:], out_offset=None, in_=lp_flat,
                in_offset=bass.IndirectOffsetOnAxis(ap=idxr_i[:], axis=0))

            oseq = mid.tile([128, T], I32, tag="oseq")
            if t_cur > 0:
                nc.vector.tensor_copy(oseq[:, 0:t_cur], psq[:, 0:t_cur])
            nc.vector.tensor_copy(oseq[:, t_cur:t_cur + 1], tok128[:])
            if t_cur + 1 < T:
                nc.vector.tensor_copy(oseq[:, t_cur + 1:], seq_sb[rt][:, t_cur + 1:])
            nc.sync.dma_start(o_seq[rt * 128:(rt + 1) * 128, :], oseq[:])

            oslp = mid.tile([128, T], F32, tag="oslp")
            if t_cur > 0:
                nc.vector.tensor_copy(oslp[:, 0:t_cur], psl[:, 0:t_cur])
            nc.vector.tensor_copy(oslp[:, t_cur:t_cur + 1], r128[:])
            if t_cur + 1 < T:
                nc.vector.tensor_copy(oslp[:, t_cur + 1:], seqlp_sb[rt][:, t_cur + 1:])
            nc.sync.dma_start(o_slp[rt * 128:(rt + 1) * 128, :], oslp[:])

    nc.finalize()
    return nc


@functools.lru_cache(maxsize=4)
def _program(t_cur: int) -> bass.Bass:
    return build_program(t_cur)


def _host_counts(prev_decisions: np.ndarray) -> np.ndarray:
    tok = prev_decisions.reshape(B, -1).astype(np.int64)
    counts = np.zeros((B, V), np.float32)
    np.add.at(counts, (np.arange(B)[:, None], tok), 1.0)
    return counts


def kernel(**inputs) -> tuple:
    logprobs = np.asarray(inputs["logprobs"], dtype=np.float32)
    beam_seq = np.asarray(inputs["beam_seq"])
    beam_seq_logprobs = np.asarray(inputs["beam_seq_logprobs"], dtype=np.float32)
    beam_logprobs_sum = np.asarray(inputs["beam_logprobs_sum"], dtype=np.float32)
    state = np.asarray(inputs["state"], dtype=np.float32)
    prev_decisions = np.asarray(inputs["prev_decisions"])
    t_cur = int(np.asarray(inputs["t"]))

    nc = _program(t_cur)
    counts = _host_counts(prev_decisions)

    # shared constant tables
    p = np.arange(128)
    bloc_np = ((p // K) * K).astype(np.float32).reshape(128, 1)
    ltm_np = np.zeros((BL, 16, 16), np.float32)
    for i in range(16):
        ltm_np[:, i, :i] = 1.0
    ltm_np = ltm_np.reshape(BL, 256)

    in_maps = []
    for c in range(NCORES):
        bsl = slice(c * BL, (c + 1) * BL)
        lp_pad = np.full((ROWS, VP), NEGF, np.float32)
        lp_pad[:, :V] = logprobs[bsl].reshape(ROWS, V)
        lp_pad[:, V - 1] -= EOSP          # unaug (EOS-penalized) raw logprobs
        # diversity penalty applied sparsely (aug): lpa = lp - 0.5*counts
        lpa = lp_pad.copy()
        lpa_v = lpa.reshape(BL, K, VP)
        for b in range(BL):
            toks = np.unique(prev_decisions[c * BL + b].reshape(-1).astype(np.int64))
            lpa_v[b, :, toks] = (lpa_v[b, :, toks]
                                 - (LAM * counts[c * BL + b, toks])[:, None])
        bsum_sh = beam_logprobs_sum[bsl]               # [BL, K]
        bsum_np = np.stack([bsum_sh[rt * BPT:(rt + 1) * BPT].reshape(128)
                            for rt in range(NT)], axis=1).astype(np.float32)
        seq_np = np.ascontiguousarray(
            beam_seq[bsl].astype(np.float32).transpose(0, 2, 1).reshape(ROWS, T))
        seqlp_np = np.ascontiguousarray(
            beam_seq_logprobs[bsl].transpose(0, 2, 1).reshape(ROWS, T))
        state_np = np.ascontiguousarray(
            state[:, :, bsl].transpose(2, 3, 0, 1, 4).reshape(ROWS, SH4))
        in_maps.append({
            "lp": lp_pad,
            "lpa": lpa,
            "bsum": bsum_np,
            "seq": seq_np,
            "seqlp": seqlp_np,
            "state": state_np,
            "bloc16": bloc_np,
            "ltm": ltm_np,
        })

    res = run_bass_kernel_spmd(nc, in_maps, core_ids=list(range(NCORES)))

    new_seq = np.zeros((B, T, K), np.float32)
    new_slp = np.zeros((B, T, K), np.float32)
    sel_p = np.zeros((B, K), np.float32)
    new_state = np.zeros((2, 2, B, K, H), np.float32)
    for c in range(NCORES):
        r = res.results[c]
        bsl = slice(c * BL, (c + 1) * BL)
        new_seq[bsl] = r["o_seq"].reshape(BL, K, T).transpose(0, 2, 1)
        new_slp[bsl] = r["o_slp"].reshape(BL, K, T).transpose(0, 2, 1)
        sel_p[bsl] = r["o_selp"].reshape(BL, K)
        new_state[:, :, bsl] = (
            r["o_state"].reshape(BL, K, 2, 2, H).transpose(2, 3, 0, 1, 4))
    return (
        new_seq.astype(beam_seq.dtype),
        new_slp.astype(np.float32),
        sel_p.astype(np.float32),
        new_state.astype(np.float32),
    )
